# revision 24
# baseline (speedup 1.0000x reference)
"""Cross-attention kernel for Trainium2, data-parallel over batch across 8 NeuronCores.

Reference computation (per batch element b):
    q = Wq @ sem_b + bq   [64, 4096]   (1x1 conv == per-pixel linear)
    k = Wk @ foren_b + bk [64, 4096]
    v = Wv @ foren_b + bv [256, 4096]
    S = (q^T k) / 8                [4096 (n), 4096 (m)]
    P = softmax_m(S)
    out = v @ P^T                  [256, 4096]
    y = sem_b + gamma * out

Kernel structure (v3 -- bf16 I/O, DMA-paced phase 1, balanced exp offload):
  - Host casts sem/foren/weights to bf16 (halves input DMA to ~4.2MB) and
    folds gamma into Wv^T and bv. Output y is written bf16 and upcast on host.
  - q/k weights are column-duplicated on the host ([W^T|W^T] -> M=128
    stationary) so projections avoid the M=64 half-rate path AND land q/k
    duplicated in both partition halves, as the 2-row-packed S matmul needs.
  - Phase 1 is a per-chunk pipeline paced by DMA: as foren chunk h lands,
    k-proj(h), v-proj(h) and the two chunk-0 S^T pairs (2h, 2h+1) run
    immediately (q2 chunk 0 is projected first from an early sem block), so
    the exp engines start ~8us into the kernel instead of after all loads.
    PSUM evacuations are split across ScalarE (k2 bias-add via
    Identity-activation, half the vt copies) and VectorE (q2 bias-add, rest)
    to keep both at or below the PE's phase-1 workload.
  - S^T tiles ([m-tile, n-chunk], m on partitions) via bf16 k2/q2: pairs of
    K=64 matmuls packed in the two PE row halves (tile_position) run
    concurrently (~one N=512 stream per pair).
  - exp: Exp(0.125*st - 3) -> fp8e4m3 pt tiles laid out [k, pair, slot, n].
    The -3 shift cancels in softmax (denominator uses the same shift).
    10 pairs/chunk on ScalarE (activation), 6 on VectorE via an fp16-bits
    Schraudolph (u16 = A*(0.125*st)+B are fp16 bit patterns of exp within
    ~3%), balancing the two engines under the PE roofline.
  - out^[c, n] = sum_m vt[m, c] * P~[m, n] via fp8 DoubleRow matmuls:
    stationary vt [Ki=128, 2, 128] (two m-tiles packed per weight load),
    moving pt [Ki=128, 2, 512] -> ~2x bf16 rate. A third "ones" column group
    accumulates the softmax denominator den[1, n] the same way.
  - rinv = 1/den broadcast to [128, n] with a K=1 outer-product matmul;
    VectorE fuses normalize (PSUM x SBUF multiply) + residual add;
    y = (sem + gamma*bv) + gamma*out_normalized. No PE transposes anywhere.
  - Phase 2 emits chunk j+1's S/exp before chunk j's out burst so the exp
    engines always have a full chunk of S tiles queued.
"""

import os
import sys

for _p in ("/opt/trn_rl_repo",):
    if _p not in sys.path and os.path.isdir(_p):
        sys.path.append(_p)

import numpy as np
import ml_dtypes

import concourse.bass as bass
import concourse.tile as tile
from concourse import bacc, mybir
from concourse.bass_utils import run_bass_kernel_spmd

N_CORES = 8
DIM = 256
D4 = 64
HW = 4096
P = 128
NCH = 512             # columns per chunk
NCHUNKS = HW // NCH   # 8
MT = HW // P          # 32 m-tiles
PAIRS = MT // 2       # 16 DoubleRow pairs
QP = 1024             # DMA block width (2KB bf16 lines)

F32 = mybir.dt.float32
BF16 = mybir.dt.bfloat16
FP8 = mybir.dt.float8e4

TRACE = False
_CACHE = {}

# fp8e4m3-bits Schraudolph exp for the DVE offload pairs (single op):
# u8 = round(A8*st + B8) are the bit patterns of fp8e4 exp(0.125*st - 3)
# to within ~7% -- same worst-case as exact-exp-then-fp8-RNE would give
# after the Schraudolph fp16 detour, but one DVE instruction instead of
# two. (3-bit mantissa -> exponent multiplier 8; bias 7 and the -3 shift
# land all values in the normal range, bits 12..30.)
A8 = 8 * 0.125 / 0.6931471805599453   # 8*log2(e)*0.125
B8 = 21.005  # 8*(7 - 3*log2(e)) - 0.367, fitted vs fp8 decode (RNE convert)
# pairs per chunk evaluated on VectorE. With 2 st PSUM buffers, S pair g
# waits on exp(g-2): buffer A serves even pairs, buffer B odd pairs. An
# all-even DVE set keeps each buffer's exp chain on (mostly) one engine,
# so the two chains drain independently instead of cross-stalling.
DVE_SET = (2, 4, 6, 8, 10, 12)


def _build_program():
    AF = mybir.ActivationFunctionType
    ALU = mybir.AluOpType
    DR = mybir.MatmulPerfMode.DoubleRow

    nc = bacc.Bacc("TRN2", target_bir_lowering=False, debug=False,
                   num_devices=N_CORES)

    sem_d = nc.dram_tensor("sem_b", [DIM, HW], BF16, kind="ExternalInput")
    foren_d = nc.dram_tensor("foren_b", [DIM, HW], BF16, kind="ExternalInput")
    wq2_d = nc.dram_tensor("wq2", [DIM, P], BF16, kind="ExternalInput")
    wk2_d = nc.dram_tensor("wk2", [DIM, P], BF16, kind="ExternalInput")
    wvt_d = nc.dram_tensor("wvt", [DIM, DIM], BF16, kind="ExternalInput")
    bq2_d = nc.dram_tensor("bq2", [P, 1], F32, kind="ExternalInput")
    bk2_d = nc.dram_tensor("bk2", [P, 1], F32, kind="ExternalInput")
    bvg_d = nc.dram_tensor("bvg", [DIM, 1], F32, kind="ExternalInput")
    y_d = nc.dram_tensor("y", [DIM, HW], BF16, kind="ExternalOutput")

    with tile.TileContext(nc) as tc:
        with (
            tc.tile_pool(name="persist", bufs=1) as persist,
            tc.tile_pool(name="ptp", bufs=2) as ptp,
            tc.tile_pool(name="ystage", bufs=2) as ystage,
            tc.tile_pool(name="small", bufs=2) as small,
            tc.tile_pool(name="st_ps", bufs=2, space="PSUM") as st_ps,
            tc.tile_pool(name="out_ps", bufs=3, space="PSUM") as out_ps,
            tc.tile_pool(name="drb_ps", bufs=1, space="PSUM") as drb_ps,
        ):
            # ---- persistent tiles ----
            fo_f = persist.tile([P, 2, HW], BF16, tag="fof")
            sem_r = persist.tile([P, 2, HW], BF16, tag="semres")
            q2 = persist.tile([P, HW], BF16, tag="q2")
            k2 = persist.tile([P, HW], BF16, tag="k2")
            vt = persist.tile([P, PAIRS, 2, DIM], FP8, tag="vt")
            wq2s = persist.tile([P, 2, P], BF16, tag="wq2s")
            wk2s = persist.tile([P, 2, P], BF16, tag="wk2s")
            wvts = persist.tile([P, 2, DIM], BF16, tag="wvts")
            bq_s = persist.tile([P, 1], F32, tag="bq")
            bk_s = persist.tile([P, 1], F32, tag="bk")
            bvg_s = persist.tile([P, 2, 1], F32, tag="bvg")
            ones2 = persist.tile([P, 2, 16], FP8, tag="ones2")
            ones1 = persist.tile([1, P], BF16, tag="ones1")
            bm3 = persist.tile([P, 1], F32, tag="bm3")

            # ---- DMA issue: order matters per queue ----
            # scalar queue: tiny bias tensors only (the scalar dynamic queue
            # is slow for bulk; weights must ride the fast queues)
            nc.scalar.dma_start(bq_s[:], bq2_d[:])
            nc.scalar.dma_start(bk_s[:], bk2_d[:])
            nc.scalar.dma_start(bvg_s[:], bvg_d.ap().rearrange("(t p) o -> p t o", p=P))
            # sync/gpsimd queues carry weights then the bulk, interleaved by
            # t-half. Order: weights (gate every projection), foren block 0
            # (gates k-proj(0)), sem block 0 (gates q-proj(0)), rest of foren
            # (gates the later chunk-0 S pairs), rest of sem.
            nc.sync.dma_start(wk2s[:], wk2_d.ap().rearrange("(t p) o -> p t o", p=P))
            nc.gpsimd.dma_start(wq2s[:], wq2_d.ap().rearrange("(t p) o -> p t o", p=P))
            nc.gpsimd.dma_start(wvts[:], wvt_d.ap().rearrange("(t p) o -> p t o", p=P))
            # The scalar queue (idle after the tiny biases) carries the last
            # foren block as a third bulk lane, pulling all-of-foren
            # residency (gates k-proj and every chunk-0 S pair) ~2us earlier.
            nq = slice(3 * QP, 4 * QP)
            nc.scalar.dma_start(fo_f[:, 0, nq], foren_d[0:P, nq])
            nc.scalar.dma_start(fo_f[:, 1, nq], foren_d[P:2 * P, nq])
            nc.sync.dma_start(fo_f[:, 0, 0:QP], foren_d[0:P, 0:QP])
            nc.gpsimd.dma_start(fo_f[:, 1, 0:QP], foren_d[P:2 * P, 0:QP])
            nc.sync.dma_start(sem_r[:, 0, 0:QP], sem_d[0:P, 0:QP])
            nc.gpsimd.dma_start(sem_r[:, 1, 0:QP], sem_d[P:2 * P, 0:QP])
            for pc_ in (1, 2):
                nq = slice(pc_ * QP, (pc_ + 1) * QP)
                nc.sync.dma_start(fo_f[:, 0, nq], foren_d[0:P, nq])
                nc.gpsimd.dma_start(fo_f[:, 1, nq], foren_d[P:2 * P, nq])
            for pc_ in range(1, HW // QP):
                nq = slice(pc_ * QP, (pc_ + 1) * QP)
                nc.sync.dma_start(sem_r[:, 0, nq], sem_d[0:P, nq])
                nc.gpsimd.dma_start(sem_r[:, 1, nq], sem_d[P:2 * P, nq])

            nc.vector.memset(ones2[:], 1.0)
            nc.vector.memset(ones1[:], 1.0)
            nc.vector.memset(bm3[:], -3.0)

            pt = [ptp.tile([P, PAIRS, 2, NCH], FP8, tag="pt", name=f"pt{t}")
                  for t in range(2)]

            def s_pair(j, g, ns):
                """S^T matmul pair g of chunk j + exp eviction into pt."""
                st = st_ps.tile([P, 2, NCH], F32, tag="st", name="st")
                m0, m1 = 2 * g, 2 * g + 1
                nc.tensor.matmul(st[:, 0, :],
                                 k2[0:D4, m0 * P:(m0 + 1) * P], q2[0:D4, ns],
                                 start=True, stop=True, tile_position=(0, 0))
                nc.tensor.matmul(st[:, 1, :],
                                 k2[D4:P, m1 * P:(m1 + 1) * P], q2[D4:P, ns],
                                 start=True, stop=True, tile_position=(64, 0))
                if g in DVE_SET:
                    nc.vector.tensor_scalar(
                        pt[j % 2][:, g, :, :].bitcast(mybir.dt.uint8),
                        st[:], A8, B8, op0=ALU.mult, op1=ALU.add)
                else:
                    nc.scalar.activation(pt[j % 2][:, g, :, :], st[:],
                                         AF.Exp, bias=bm3[:], scale=0.125)

            # ---- phase 1: DMA-paced per-chunk pipeline ----
            for h in range(NCHUNKS):
                ns = slice(h * NCH, (h + 1) * NCH)
                # k-projection for chunk h
                pk = out_ps.tile([P, NCH], F32, tag="out", name="pk")
                for t in range(2):
                    nc.tensor.matmul(pk[:], wk2s[:, t, :], fo_f[:, t, ns],
                                     start=(t == 0), stop=(t == 1))
                nc.scalar.activation(k2[:, ns], pk[:], AF.Identity,
                                     bias=bk_s[:])
                if h == 0:
                    # q-projection chunk 0 (gates every chunk-0 S pair)
                    pq = out_ps.tile([P, NCH], F32, tag="out", name="pq")
                    for t in range(2):
                        nc.tensor.matmul(pq[:], wq2s[:, t, :],
                                         sem_r[:, t, 0:NCH],
                                         start=(t == 0), stop=(t == 1))
                    nc.vector.tensor_scalar_add(q2[:, 0:NCH], pq[:], bq_s[:])
                # v-projection: two m-tiles share one PSUM bank, one evac
                for mp in (4 * h, 4 * h + 2):
                    pv = out_ps.tile([P, NCH], F32, tag="out", name="pv")
                    for sl in range(2):
                        mi = mp + sl
                        c0 = h * NCH + (mi - 4 * h) * P
                        for t in range(2):
                            nc.tensor.matmul(pv[:, sl * DIM:(sl + 1) * DIM],
                                             fo_f[:, t, c0:c0 + P],
                                             wvts[:, t, :],
                                             start=(t == 0), stop=(t == 1))
                    if mp % 4 == 0:
                        nc.vector.tensor_copy(vt[:, mp // 2, :, :], pv[:])
                    else:
                        nc.scalar.copy(vt[:, mp // 2, :, :], pv[:])
                # chunk-0 S pairs for the two m-tile pairs this chunk enables
                s_pair(0, 2 * h, slice(0, NCH))
                s_pair(0, 2 * h + 1, slice(0, NCH))
            for h in range(1, NCHUNKS):
                ns = slice(h * NCH, (h + 1) * NCH)
                pq = out_ps.tile([P, NCH], F32, tag="out", name="pq")
                for t in range(2):
                    nc.tensor.matmul(pq[:], wq2s[:, t, :], sem_r[:, t, ns],
                                     start=(t == 0), stop=(t == 1))
                nc.vector.tensor_scalar_add(q2[:, ns], pq[:], bq_s[:])

            # ---- phase 2: attention chunks ----
            # The PE instruction stream interleaves chunk j+1's S pairs with
            # chunk j's out-groups: each S matmul must wait for an st PSUM
            # bank (freed at exp cadence, ~1.1us), and with a strict-FIFO PE
            # queue an all-S-then-all-out order leaves the PE blocked at the
            # queue head for most of the exp span. Slotting one out-group
            # (3 ready-to-run DR matmuls, ~0.72us) between consecutive S
            # pairs keeps the PE busy while the st rotation catches up.
            # Per iteration j: (a) two plain DVE casts at the head of the
            # Vector queue evacuate chunk j-1's out accumulators to SBUF
            # (frees the PSUM buffers chunk j's first out-groups need within
            # ~1.4us of chunk start), (b) DVE computes chunk j-1's rinv +
            # normalize from SBUF, (c) the PE stream interleaves chunk j's
            # out-groups with chunk j+1's S pairs (each S matmul waits on an
            # st PSUM bank freed at exp cadence; slotting one ready-to-run
            # out-group between S pairs keeps the strict-FIFO PE queue busy).
            order = ([g for g in range(PAIRS) if g not in DVE_SET]
                     + list(DVE_SET))
            prev = None  # (out0, out1, den, ns) of chunk j-1

            def finish_chunk(prev):
                out0p, out1p, drb, nsp = prev
                out0s = ystage.tile([P, NCH], BF16, tag="o0s", name="out0s")
                out1s = ystage.tile([P, NCH], BF16, tag="o1s", name="out1s")
                nc.vector.tensor_copy(out0s[:], out0p[:])
                nc.vector.tensor_copy(out1s[:], out1p[:])
                rrf = small.tile([1, NCH], F32, tag="rrf", name="rrf")
                nc.vector.reciprocal_approx_fast(rrf[:], drb[0:1, :])
                rr = small.tile([1, NCH], BF16, tag="rr", name="rr")
                nc.vector.tensor_copy(rr[:], rrf[:])
                # rinv broadcast reuses the den bank: the rank-1 matmul's
                # write of the full [P, NCH] region overlaps den's [0:1, :]
                # slice, so Tile orders it after the reciprocal's read, and
                # orders the next chunk's den accumulation after the rb cast.
                nc.tensor.matmul(drb[:], ones1[:], rr[:], start=True, stop=True)
                rb = small.tile([P, NCH], BF16, tag="rbs", name="rb")
                nc.vector.tensor_copy(rb[:], drb[:])
                yst = ystage.tile([P, 2, NCH], BF16, tag="yst", name="yst")
                for ct, outp in ((0, out0s), (1, out1s)):
                    nc.vector.tensor_tensor(yst[:, ct, :], outp[:], rb[:],
                                            op=mybir.AluOpType.mult)
                    nc.vector.scalar_tensor_tensor(yst[:, ct, :], yst[:, ct, :],
                                                   bvg_s[:, ct, :],
                                                   sem_r[:, ct, nsp],
                                                   op0=mybir.AluOpType.add,
                                                   op1=mybir.AluOpType.add)
                    nc.sync.dma_start(y_d[ct * P:(ct + 1) * P, nsp],
                                      yst[:, ct, :])

            for j in range(NCHUNKS):
                ns = slice(j * NCH, (j + 1) * NCH)
                ns1 = slice((j + 1) * NCH, (j + 2) * NCH)
                out0 = out_ps.tile([P, NCH], F32, tag="out", name="out0")
                out1 = out_ps.tile([P, NCH], F32, tag="out", name="out1")
                drb = drb_ps.tile([P, NCH], F32, tag="drb", name="drb")

                def out_group(idx):
                    g = order[idx]
                    pslice = pt[j % 2][:, g, :, :]
                    first, last = idx == 0, idx == PAIRS - 1
                    nc.tensor.matmul(out0[:], vt[:, g, :, 0:P], pslice,
                                     start=first, stop=last, perf_mode=DR)
                    nc.tensor.matmul(out1[:], vt[:, g, :, P:DIM], pslice,
                                     start=first, stop=last, perf_mode=DR)
                    nc.tensor.matmul(drb[0:1, :], ones2[:, :, 0:1], pslice,
                                     start=first, stop=last, perf_mode=DR)

                # S pairs clustered in twos: each out<->S transition on the
                # PE costs ~100ns (the row-packed S pair blocks the next
                # LDWEIGHTS pull-ahead), so fewer, larger clusters beat a
                # strict 1:1 interleave.
                HEAD = 2
                if j + 1 < NCHUNKS:
                    for g in range(HEAD):
                        s_pair(j + 1, g, ns1)
                        if g == 1 and prev is not None:
                            finish_chunk(prev)
                    for g in range(HEAD, PAIRS, 2):
                        out_group(g - HEAD)
                        out_group(g - HEAD + 1)
                        s_pair(j + 1, g, ns1)
                        s_pair(j + 1, g + 1, ns1)
                    for idx in range(PAIRS - HEAD, PAIRS):
                        out_group(idx)
                else:
                    if prev is not None:
                        finish_chunk(prev)
                    for idx in range(PAIRS):
                        out_group(idx)
                prev = (out0, out1, drb, ns)
            finish_chunk(prev)

    nc.compile()
    return nc


def _get_program():
    if "nc" not in _CACHE:
        _CACHE["nc"] = _build_program()
    return _CACHE["nc"]


def kernel(sem, foren, Wq, bq, Wk, bk, Wv, bv, gamma):
    BF = ml_dtypes.bfloat16
    sem = np.asarray(sem, dtype=np.float32)
    foren = np.asarray(foren, dtype=np.float32)
    wqt = np.asarray(Wq, np.float32).T          # [256, 64]
    wkt = np.asarray(Wk, np.float32).T
    g = float(np.asarray(gamma, np.float32).reshape(()))
    wvtg = np.ascontiguousarray(g * np.asarray(Wv, np.float32).T).astype(BF)
    wq2 = np.ascontiguousarray(np.concatenate([wqt, wqt], axis=1)).astype(BF)
    wk2 = np.ascontiguousarray(np.concatenate([wkt, wkt], axis=1)).astype(BF)
    bqv = np.asarray(bq, np.float32).reshape(D4, 1)
    bkv = np.asarray(bk, np.float32).reshape(D4, 1)
    bq2 = np.ascontiguousarray(np.tile(bqv, (2, 1)))
    bk2 = np.ascontiguousarray(np.tile(bkv, (2, 1)))
    bvg = np.ascontiguousarray(g * np.asarray(bv, np.float32).reshape(DIM, 1))

    B = sem.shape[0]
    assert B == N_CORES, f"expected batch {N_CORES}, got {B}"

    in_maps = []
    for i in range(N_CORES):
        in_maps.append({
            "sem_b": np.ascontiguousarray(sem[i].reshape(DIM, HW)).astype(BF),
            "foren_b": np.ascontiguousarray(foren[i].reshape(DIM, HW)).astype(BF),
            "wq2": wq2, "wk2": wk2, "wvt": wvtg,
            "bq2": bq2, "bk2": bk2, "bvg": bvg,
        })

    nc = _get_program()
    res = run_bass_kernel_spmd(nc, in_maps, list(range(N_CORES)), trace=TRACE)
    if TRACE:
        _CACHE["last_exec_time_ns"] = res.exec_time_ns
        _CACHE["last_results"] = res

    H = int(np.sqrt(HW))
    out = np.stack([np.asarray(res.results[i]["y"]).astype(np.float32)
                    .reshape(DIM, H, H) for i in range(N_CORES)])
    return out


# revision 25
# speedup vs baseline: 1.0117x; 1.0117x over previous
"""Cross-attention kernel for Trainium2, data-parallel over batch across 8 NeuronCores.

Reference computation (per batch element b):
    q = Wq @ sem_b + bq   [64, 4096]   (1x1 conv == per-pixel linear)
    k = Wk @ foren_b + bk [64, 4096]
    v = Wv @ foren_b + bv [256, 4096]
    S = (q^T k) / 8                [4096 (n), 4096 (m)]
    P = softmax_m(S)
    out = v @ P^T                  [256, 4096]
    y = sem_b + gamma * out

Kernel structure (bf16 I/O, DMA-paced phase 1, interleaved phase 2):
  - Host casts sem/foren/weights to bf16 (halves input DMA to ~4.2MB) and
    folds gamma into Wv^T and bv. Output y is written bf16 and upcast on host.
  - q/k weights are column-duplicated on the host ([W^T|W^T] -> M=128
    stationary) so projections avoid the M=64 half-rate path AND land q/k
    duplicated in both partition halves, as the 2-row-packed S matmul needs.
  - Phase 1 is a per-chunk pipeline paced by DMA: as foren chunk h lands,
    k-proj(h), v-proj(h) and the two chunk-0 S^T pairs (2h, 2h+1) run
    immediately (q2 chunk 0 is projected first from an early sem block), so
    the exp engines start ~16us into the kernel instead of after all loads.
    PSUM evacuations are split across ScalarE (k2 bias-add via
    Identity-activation, half the vt copies) and VectorE (q2 bias-add, rest).
  - S^T tiles ([m-tile, n-chunk], m on partitions) via bf16 k2/q2: pairs of
    K=64 matmuls packed in the two PE row halves (tile_position) run
    concurrently (~one N=512 stream per pair).
  - exp: Exp(0.125*st - 3) -> fp8e4m3 pt tiles laid out [k, pair, slot, n].
    The -3 shift cancels in softmax (denominator uses the same shift).
    10 pairs/chunk on ScalarE (activation); 6 on VectorE via a single
    tensor_scalar producing fp8e4 BIT PATTERNS directly (Schraudolph in the
    3-bit-mantissa domain, ~7% worst case -- same order as exp->fp8 RNE).
  - out^[c, n] = sum_m vt[m, c] * P~[m, n] via fp8 DoubleRow matmuls:
    stationary vt [Ki=128, 2, 128] (two m-tiles packed per weight load),
    moving pt [Ki=128, 2, 512] -> 2x bf16 rate (216ns/matmul measured).
    A third "ones" group accumulates the softmax denominator den[1, n] into
    partition 0 of a bank shared with the rinv broadcast (Tile's slice
    tracking orders the overlap), freeing a PSUM bank for a 3rd out buffer.
  - Phase 2 PE stream interleaves chunk j+1's S pairs (gated by the 2-buffer
    st rotation at exp cadence) with chunk j's always-ready out-groups in a
    2:2 pattern, so the strict-FIFO PE queue never idles at a blocked S
    matmul. DVE_SET = even pairs keeps each st buffer's exp chain on one
    engine. Chunk j-1's out accumulators are evacuated to SBUF by two DVE
    casts at the head of the chunk's Vector queue; normalize + residual run
    from SBUF (y = (sem + gamma*bv) + gamma*out*rinv). No PE transposes.
"""

import os
import sys

for _p in ("/opt/trn_rl_repo",):
    if _p not in sys.path and os.path.isdir(_p):
        sys.path.append(_p)

import numpy as np
import ml_dtypes

import concourse.bass as bass
import concourse.tile as tile
from concourse import bacc, mybir
from concourse.bass_utils import run_bass_kernel_spmd

N_CORES = 8
DIM = 256
D4 = 64
HW = 4096
P = 128
NCH = 512             # columns per chunk
NCHUNKS = HW // NCH   # 8
MT = HW // P          # 32 m-tiles
PAIRS = MT // 2       # 16 DoubleRow pairs
QP = 1024             # DMA block width (2KB bf16 lines)

F32 = mybir.dt.float32
BF16 = mybir.dt.bfloat16
FP8 = mybir.dt.float8e4

TRACE = False
_CACHE = {}

# fp8e4m3-bits Schraudolph exp for the DVE offload pairs (single op):
# u8 = round(A8*st + B8) are the bit patterns of fp8e4 exp(0.125*st - 3)
# to within ~7% -- same worst-case as exact-exp-then-fp8-RNE would give
# after the Schraudolph fp16 detour, but one DVE instruction instead of
# two. (3-bit mantissa -> exponent multiplier 8; bias 7 and the -3 shift
# land all values in the normal range, bits 12..30.)
A8 = 8 * 0.125 / 0.6931471805599453   # 8*log2(e)*0.125
B8 = 21.005  # 8*(7 - 3*log2(e)) - 0.367, fitted vs fp8 decode (RNE convert)
# pairs per chunk evaluated on VectorE. With 2 st PSUM buffers, S pair g
# waits on exp(g-2): buffer A serves even pairs, buffer B odd pairs. An
# all-even DVE set keeps each buffer's exp chain on (mostly) one engine,
# so the two chains drain independently instead of cross-stalling.
DVE_SET = (2, 4, 6, 8, 10, 12)


def _build_program():
    AF = mybir.ActivationFunctionType
    ALU = mybir.AluOpType
    DR = mybir.MatmulPerfMode.DoubleRow

    nc = bacc.Bacc("TRN2", target_bir_lowering=False, debug=False,
                   num_devices=N_CORES)

    sem_d = nc.dram_tensor("sem_b", [DIM, HW], BF16, kind="ExternalInput")
    foren_d = nc.dram_tensor("foren_b", [DIM, HW], BF16, kind="ExternalInput")
    wq2_d = nc.dram_tensor("wq2", [DIM, P], BF16, kind="ExternalInput")
    wk2_d = nc.dram_tensor("wk2", [DIM, P], BF16, kind="ExternalInput")
    wvt_d = nc.dram_tensor("wvt", [DIM, DIM], BF16, kind="ExternalInput")
    bq2_d = nc.dram_tensor("bq2", [P, 1], F32, kind="ExternalInput")
    bk2_d = nc.dram_tensor("bk2", [P, 1], F32, kind="ExternalInput")
    bvg_d = nc.dram_tensor("bvg", [DIM, 1], F32, kind="ExternalInput")
    y_d = nc.dram_tensor("y", [DIM, HW], BF16, kind="ExternalOutput")

    with tile.TileContext(nc) as tc:
        with (
            tc.tile_pool(name="persist", bufs=1) as persist,
            tc.tile_pool(name="ptp", bufs=2) as ptp,
            tc.tile_pool(name="ystage", bufs=2) as ystage,
            tc.tile_pool(name="small", bufs=2) as small,
            tc.tile_pool(name="st_ps", bufs=2, space="PSUM") as st_ps,
            tc.tile_pool(name="out_ps", bufs=3, space="PSUM") as out_ps,
            tc.tile_pool(name="drb_ps", bufs=1, space="PSUM") as drb_ps,
        ):
            # ---- persistent tiles ----
            fo_f = persist.tile([P, 2, HW], BF16, tag="fof")
            sem_r = persist.tile([P, 2, HW], BF16, tag="semres")
            q2 = persist.tile([P, HW], BF16, tag="q2")
            k2 = persist.tile([P, HW], BF16, tag="k2")
            vt = persist.tile([P, PAIRS, 2, DIM], FP8, tag="vt")
            wq2s = persist.tile([P, 2, P], BF16, tag="wq2s")
            wk2s = persist.tile([P, 2, P], BF16, tag="wk2s")
            wvts = persist.tile([P, 2, DIM], BF16, tag="wvts")
            bq_s = persist.tile([P, 1], F32, tag="bq")
            bk_s = persist.tile([P, 1], F32, tag="bk")
            bvg_s = persist.tile([P, 2, 1], F32, tag="bvg")
            ones2 = persist.tile([P, 2, 16], FP8, tag="ones2")
            ones1 = persist.tile([1, P], BF16, tag="ones1")
            bm3 = persist.tile([P, 1], F32, tag="bm3")

            # ---- DMA issue: order matters per queue ----
            # scalar queue: tiny bias tensors only (the scalar dynamic queue
            # is slow for bulk; weights must ride the fast queues)
            nc.scalar.dma_start(bq_s[:], bq2_d[:])
            nc.scalar.dma_start(bk_s[:], bk2_d[:])
            nc.scalar.dma_start(bvg_s[:], bvg_d.ap().rearrange("(t p) o -> p t o", p=P))
            # sync/gpsimd queues carry weights then the bulk, interleaved by
            # t-half. Order: weights (gate every projection), foren block 0
            # (gates k-proj(0)), sem block 0 (gates q-proj(0)), rest of foren
            # (gates the later chunk-0 S pairs), rest of sem.
            nc.sync.dma_start(wk2s[:], wk2_d.ap().rearrange("(t p) o -> p t o", p=P))
            nc.gpsimd.dma_start(wq2s[:], wq2_d.ap().rearrange("(t p) o -> p t o", p=P))
            nc.gpsimd.dma_start(wvts[:], wvt_d.ap().rearrange("(t p) o -> p t o", p=P))
            # The scalar queue (idle after the tiny biases) carries the last
            # foren block as a third bulk lane, pulling all-of-foren
            # residency (gates k-proj and every chunk-0 S pair) ~2us earlier.
            nq = slice(3 * QP, 4 * QP)
            nc.scalar.dma_start(fo_f[:, 0, nq], foren_d[0:P, nq])
            nc.scalar.dma_start(fo_f[:, 1, nq], foren_d[P:2 * P, nq])
            nc.sync.dma_start(fo_f[:, 0, 0:QP], foren_d[0:P, 0:QP])
            nc.gpsimd.dma_start(fo_f[:, 1, 0:QP], foren_d[P:2 * P, 0:QP])
            nc.sync.dma_start(sem_r[:, 0, 0:QP], sem_d[0:P, 0:QP])
            nc.gpsimd.dma_start(sem_r[:, 1, 0:QP], sem_d[P:2 * P, 0:QP])
            for pc_ in (1, 2):
                nq = slice(pc_ * QP, (pc_ + 1) * QP)
                nc.sync.dma_start(fo_f[:, 0, nq], foren_d[0:P, nq])
                nc.gpsimd.dma_start(fo_f[:, 1, nq], foren_d[P:2 * P, nq])
            for pc_ in range(1, HW // QP):
                nq = slice(pc_ * QP, (pc_ + 1) * QP)
                nc.sync.dma_start(sem_r[:, 0, nq], sem_d[0:P, nq])
                nc.gpsimd.dma_start(sem_r[:, 1, nq], sem_d[P:2 * P, nq])

            nc.vector.memset(ones2[:], 1.0)
            nc.vector.memset(ones1[:], 1.0)
            nc.vector.memset(bm3[:], -3.0)

            pt = [ptp.tile([P, PAIRS, 2, NCH], FP8, tag="pt", name=f"pt{t}")
                  for t in range(2)]

            def s_pair(j, g, ns):
                """S^T matmul pair g of chunk j + exp eviction into pt."""
                st = st_ps.tile([P, 2, NCH], F32, tag="st", name="st")
                m0, m1 = 2 * g, 2 * g + 1
                nc.tensor.matmul(st[:, 0, :],
                                 k2[0:D4, m0 * P:(m0 + 1) * P], q2[0:D4, ns],
                                 start=True, stop=True, tile_position=(0, 0))
                nc.tensor.matmul(st[:, 1, :],
                                 k2[D4:P, m1 * P:(m1 + 1) * P], q2[D4:P, ns],
                                 start=True, stop=True, tile_position=(64, 0))
                if g in DVE_SET:
                    nc.vector.tensor_scalar(
                        pt[j % 2][:, g, :, :].bitcast(mybir.dt.uint8),
                        st[:], A8, B8, op0=ALU.mult, op1=ALU.add)
                else:
                    nc.scalar.activation(pt[j % 2][:, g, :, :], st[:],
                                         AF.Exp, bias=bm3[:], scale=0.125)

            # ---- phase 1: DMA-paced per-chunk pipeline ----
            for h in range(NCHUNKS):
                ns = slice(h * NCH, (h + 1) * NCH)
                # k-projection for chunk h
                pk = out_ps.tile([P, NCH], F32, tag="out", name="pk")
                for t in range(2):
                    nc.tensor.matmul(pk[:], wk2s[:, t, :], fo_f[:, t, ns],
                                     start=(t == 0), stop=(t == 1))
                nc.scalar.activation(k2[:, ns], pk[:], AF.Identity,
                                     bias=bk_s[:])
                if h == 0:
                    # q-projection chunk 0 (gates every chunk-0 S pair)
                    pq = out_ps.tile([P, NCH], F32, tag="out", name="pq")
                    for t in range(2):
                        nc.tensor.matmul(pq[:], wq2s[:, t, :],
                                         sem_r[:, t, 0:NCH],
                                         start=(t == 0), stop=(t == 1))
                    nc.vector.tensor_scalar_add(q2[:, 0:NCH], pq[:], bq_s[:])
                # v-projection: two m-tiles share one PSUM bank, one evac
                for mp in (4 * h, 4 * h + 2):
                    pv = out_ps.tile([P, NCH], F32, tag="out", name="pv")
                    for sl in range(2):
                        mi = mp + sl
                        c0 = h * NCH + (mi - 4 * h) * P
                        for t in range(2):
                            nc.tensor.matmul(pv[:, sl * DIM:(sl + 1) * DIM],
                                             fo_f[:, t, c0:c0 + P],
                                             wvts[:, t, :],
                                             start=(t == 0), stop=(t == 1))
                    if mp % 4 == 0:
                        nc.vector.tensor_copy(vt[:, mp // 2, :, :], pv[:])
                    else:
                        nc.scalar.copy(vt[:, mp // 2, :, :], pv[:])
                # chunk-0 S pairs for the two m-tile pairs this chunk enables
                s_pair(0, 2 * h, slice(0, NCH))
                s_pair(0, 2 * h + 1, slice(0, NCH))
            for h in range(1, NCHUNKS):
                ns = slice(h * NCH, (h + 1) * NCH)
                pq = out_ps.tile([P, NCH], F32, tag="out", name="pq")
                for t in range(2):
                    nc.tensor.matmul(pq[:], wq2s[:, t, :], sem_r[:, t, ns],
                                     start=(t == 0), stop=(t == 1))
                nc.vector.tensor_scalar_add(q2[:, ns], pq[:], bq_s[:])

            # ---- phase 2: attention chunks ----
            # The PE instruction stream interleaves chunk j+1's S pairs with
            # chunk j's out-groups: each S matmul must wait for an st PSUM
            # bank (freed at exp cadence, ~1.1us), and with a strict-FIFO PE
            # queue an all-S-then-all-out order leaves the PE blocked at the
            # queue head for most of the exp span. Slotting one out-group
            # (3 ready-to-run DR matmuls, ~0.72us) between consecutive S
            # pairs keeps the PE busy while the st rotation catches up.
            # Per iteration j: (a) two plain DVE casts at the head of the
            # Vector queue evacuate chunk j-1's out accumulators to SBUF
            # (frees the PSUM buffers chunk j's first out-groups need within
            # ~1.4us of chunk start), (b) DVE computes chunk j-1's rinv +
            # normalize from SBUF, (c) the PE stream interleaves chunk j's
            # out-groups with chunk j+1's S pairs (each S matmul waits on an
            # st PSUM bank freed at exp cadence; slotting one ready-to-run
            # out-group between S pairs keeps the strict-FIFO PE queue busy).
            order = ([g for g in range(PAIRS) if g not in DVE_SET]
                     + list(DVE_SET))
            prev = None  # (out0, out1, den, ns) of chunk j-1

            def finish_chunk(prev):
                out0p, out1p, drb, nsp = prev
                out0s = ystage.tile([P, NCH], BF16, tag="o0s", name="out0s")
                out1s = ystage.tile([P, NCH], BF16, tag="o1s", name="out1s")
                nc.vector.tensor_copy(out0s[:], out0p[:])
                nc.vector.tensor_copy(out1s[:], out1p[:])
                rrf = small.tile([1, NCH], F32, tag="rrf", name="rrf")
                nc.vector.reciprocal_approx_fast(rrf[:], drb[0:1, :])
                rr = small.tile([1, NCH], BF16, tag="rr", name="rr")
                nc.vector.tensor_copy(rr[:], rrf[:])
                # rinv broadcast reuses the den bank: the rank-1 matmul's
                # write of the full [P, NCH] region overlaps den's [0:1, :]
                # slice, so Tile orders it after the reciprocal's read, and
                # orders the next chunk's den accumulation after the rb cast.
                nc.tensor.matmul(drb[:], ones1[:], rr[:], start=True, stop=True)
                rb = small.tile([P, NCH], BF16, tag="rbs", name="rb")
                nc.vector.tensor_copy(rb[:], drb[:])
                yst = ystage.tile([P, 2, NCH], BF16, tag="yst", name="yst")
                for ct, outp in ((0, out0s), (1, out1s)):
                    nc.vector.tensor_tensor(yst[:, ct, :], outp[:], rb[:],
                                            op=mybir.AluOpType.mult)
                    nc.vector.scalar_tensor_tensor(yst[:, ct, :], yst[:, ct, :],
                                                   bvg_s[:, ct, :],
                                                   sem_r[:, ct, nsp],
                                                   op0=mybir.AluOpType.add,
                                                   op1=mybir.AluOpType.add)
                    nc.sync.dma_start(y_d[ct * P:(ct + 1) * P, nsp],
                                      yst[:, ct, :])

            for j in range(NCHUNKS):
                ns = slice(j * NCH, (j + 1) * NCH)
                ns1 = slice((j + 1) * NCH, (j + 2) * NCH)
                out0 = out_ps.tile([P, NCH], F32, tag="out", name="out0")
                out1 = out_ps.tile([P, NCH], F32, tag="out", name="out1")
                drb = drb_ps.tile([P, NCH], F32, tag="drb", name="drb")

                def out_group(idx):
                    g = order[idx]
                    pslice = pt[j % 2][:, g, :, :]
                    first, last = idx == 0, idx == PAIRS - 1
                    nc.tensor.matmul(out0[:], vt[:, g, :, 0:P], pslice,
                                     start=first, stop=last, perf_mode=DR)
                    nc.tensor.matmul(out1[:], vt[:, g, :, P:DIM], pslice,
                                     start=first, stop=last, perf_mode=DR)
                    nc.tensor.matmul(drb[0:1, :], ones2[:, :, 0:1], pslice,
                                     start=first, stop=last, perf_mode=DR)

                # S pairs clustered in twos: each out<->S transition on the
                # PE costs ~100ns (the row-packed S pair blocks the next
                # LDWEIGHTS pull-ahead), so fewer, larger clusters beat a
                # strict 1:1 interleave.
                HEAD = 2
                if j + 1 < NCHUNKS:
                    for g in range(HEAD):
                        s_pair(j + 1, g, ns1)
                        if g == 1 and prev is not None:
                            finish_chunk(prev)
                    for g in range(HEAD, PAIRS, 2):
                        out_group(g - HEAD)
                        out_group(g - HEAD + 1)
                        s_pair(j + 1, g, ns1)
                        s_pair(j + 1, g + 1, ns1)
                    for idx in range(PAIRS - HEAD, PAIRS):
                        out_group(idx)
                else:
                    if prev is not None:
                        finish_chunk(prev)
                    for idx in range(PAIRS):
                        out_group(idx)
                prev = (out0, out1, drb, ns)
            finish_chunk(prev)

    nc.compile()
    return nc


def _get_program():
    if "nc" not in _CACHE:
        _CACHE["nc"] = _build_program()
    return _CACHE["nc"]


def kernel(sem, foren, Wq, bq, Wk, bk, Wv, bv, gamma):
    BF = ml_dtypes.bfloat16
    sem = np.asarray(sem, dtype=np.float32)
    foren = np.asarray(foren, dtype=np.float32)
    wqt = np.asarray(Wq, np.float32).T          # [256, 64]
    wkt = np.asarray(Wk, np.float32).T
    g = float(np.asarray(gamma, np.float32).reshape(()))
    wvtg = np.ascontiguousarray(g * np.asarray(Wv, np.float32).T).astype(BF)
    wq2 = np.ascontiguousarray(np.concatenate([wqt, wqt], axis=1)).astype(BF)
    wk2 = np.ascontiguousarray(np.concatenate([wkt, wkt], axis=1)).astype(BF)
    bqv = np.asarray(bq, np.float32).reshape(D4, 1)
    bkv = np.asarray(bk, np.float32).reshape(D4, 1)
    bq2 = np.ascontiguousarray(np.tile(bqv, (2, 1)))
    bk2 = np.ascontiguousarray(np.tile(bkv, (2, 1)))
    bvg = np.ascontiguousarray(g * np.asarray(bv, np.float32).reshape(DIM, 1))

    B = sem.shape[0]
    assert B == N_CORES, f"expected batch {N_CORES}, got {B}"

    in_maps = []
    for i in range(N_CORES):
        in_maps.append({
            "sem_b": np.ascontiguousarray(sem[i].reshape(DIM, HW)).astype(BF),
            "foren_b": np.ascontiguousarray(foren[i].reshape(DIM, HW)).astype(BF),
            "wq2": wq2, "wk2": wk2, "wvt": wvtg,
            "bq2": bq2, "bk2": bk2, "bvg": bvg,
        })

    nc = _get_program()
    res = run_bass_kernel_spmd(nc, in_maps, list(range(N_CORES)), trace=TRACE)
    if TRACE:
        _CACHE["last_exec_time_ns"] = res.exec_time_ns
        _CACHE["last_results"] = res

    H = int(np.sqrt(HW))
    out = np.stack([np.asarray(res.results[i]["y"]).astype(np.float32)
                    .reshape(DIM, H, H) for i in range(N_CORES)])
    return out


# revision 32
# speedup vs baseline: 1.0193x; 1.0076x over previous
"""Cross-attention kernel for Trainium2, data-parallel over batch across 8 NeuronCores.

Reference computation (per batch element b):
    q = Wq @ sem_b + bq   [64, 4096]   (1x1 conv == per-pixel linear)
    k = Wk @ foren_b + bk [64, 4096]
    v = Wv @ foren_b + bv [256, 4096]
    S = (q^T k) / 8                [4096 (n), 4096 (m)]
    P = softmax_m(S)
    out = v @ P^T                  [256, 4096]
    y = sem_b + gamma * out

Kernel structure (bf16 I/O, DMA-paced phase 1, interleaved phase 2):
  - Host casts sem/foren/weights to bf16 (halves input DMA to ~4.2MB) and
    folds gamma into Wv^T and bv. Output y is written bf16 and upcast on host.
  - q/k weights are column-duplicated on the host ([W^T|W^T] -> M=128
    stationary) so projections avoid the M=64 half-rate path AND land q/k
    duplicated in both partition halves, as the 2-row-packed S matmul needs.
  - Phase 1 is a per-chunk pipeline paced by DMA: as foren chunk h lands,
    k-proj(h), v-proj(h) and the two chunk-0 S^T pairs (2h, 2h+1) run
    immediately (q2 chunk 0 is projected first from an early sem block), so
    the exp engines start ~16us into the kernel instead of after all loads.
    PSUM evacuations are split across ScalarE (k2 bias-add via
    Identity-activation, half the vt copies) and VectorE (q2 bias-add, rest).
  - S^T tiles ([m-tile, n-chunk], m on partitions) via bf16 k2/q2: pairs of
    K=64 matmuls packed in the two PE row halves (tile_position) run
    concurrently (~one N=512 stream per pair).
  - exp: Exp(0.125*st - 3) -> fp8e4m3 pt tiles laid out [k, pair, slot, n].
    The -3 shift cancels in softmax (denominator uses the same shift).
    10 pairs/chunk on ScalarE (activation); 6 on VectorE via a single
    tensor_scalar producing fp8e4 BIT PATTERNS directly (Schraudolph in the
    3-bit-mantissa domain, ~7% worst case -- same order as exp->fp8 RNE).
  - out^[c, n] = sum_m vt[m, c] * P~[m, n] via fp8 DoubleRow matmuls:
    stationary vt [Ki=128, 2, 128] (two m-tiles packed per weight load),
    moving pt [Ki=128, 2, 512] -> 2x bf16 rate (216ns/matmul measured).
    A third "ones" group accumulates the softmax denominator den[1, n] into
    partition 0 of a bank shared with the rinv broadcast (Tile's slice
    tracking orders the overlap), freeing a PSUM bank for a 3rd out buffer.
  - Phase 2 PE stream interleaves chunk j+1's S pairs (gated by the 2-buffer
    st rotation at exp cadence) with chunk j's always-ready out-groups in a
    2:2 pattern, so the strict-FIFO PE queue never idles at a blocked S
    matmul. DVE_SET = even pairs keeps each st buffer's exp chain on one
    engine. Chunk j-1's out accumulators are evacuated to SBUF by two DVE
    casts at the head of the chunk's Vector queue; normalize + residual run
    from SBUF (y = (sem + gamma*bv) + gamma*out*rinv). No PE transposes.
"""

import os
import sys

for _p in ("/opt/trn_rl_repo",):
    if _p not in sys.path and os.path.isdir(_p):
        sys.path.append(_p)

import numpy as np
import ml_dtypes

import concourse.bass as bass
import concourse.tile as tile
from concourse import bacc, mybir
from concourse.bass_utils import run_bass_kernel_spmd

N_CORES = 8
DIM = 256
D4 = 64
HW = 4096
P = 128
NCH = 512             # columns per chunk
NCHUNKS = HW // NCH   # 8
MT = HW // P          # 32 m-tiles
PAIRS = MT // 2       # 16 DoubleRow pairs
QP = 1024             # DMA block width (2KB bf16 lines)

F32 = mybir.dt.float32
BF16 = mybir.dt.bfloat16
FP8 = mybir.dt.float8e4

TRACE = False
_CACHE = {}

# fp8e4m3-bits Schraudolph exp for the DVE offload pairs (single op):
# u8 = round(A8*st + B8) are the bit patterns of fp8e4 exp(0.125*st - 3)
# to within ~7% -- same worst-case as exact-exp-then-fp8-RNE would give
# after the Schraudolph fp16 detour, but one DVE instruction instead of
# two. (3-bit mantissa -> exponent multiplier 8; bias 7 and the -3 shift
# land all values in the normal range, bits 12..30.)
A8 = 8 * 0.125 / 0.6931471805599453   # 8*log2(e)*0.125
B8 = 21.005  # 8*(7 - 3*log2(e)) - 0.367, fitted vs fp8 decode (RNE convert)
# pairs per chunk evaluated on VectorE. With 2 st PSUM buffers, S pair g
# waits on exp(g-2): buffer A serves even pairs, buffer B odd pairs. An
# all-even DVE set keeps each buffer's exp chain on (mostly) one engine,
# so the two chains drain independently instead of cross-stalling.
DVE_SET = (2, 4, 6, 8, 10, 12)


def _build_program():
    AF = mybir.ActivationFunctionType
    ALU = mybir.AluOpType
    DR = mybir.MatmulPerfMode.DoubleRow

    nc = bacc.Bacc("TRN2", target_bir_lowering=False, debug=False,
                   num_devices=N_CORES)

    sem_d = nc.dram_tensor("sem_b", [DIM, HW], BF16, kind="ExternalInput")
    foren_d = nc.dram_tensor("foren_b", [DIM, HW], FP8, kind="ExternalInput")
    wq2_d = nc.dram_tensor("wq2", [DIM, P], BF16, kind="ExternalInput")
    wk2_d = nc.dram_tensor("wk2", [DIM, P], FP8, kind="ExternalInput")
    wvt_d = nc.dram_tensor("wvt", [DIM, DIM], FP8, kind="ExternalInput")
    bq2_d = nc.dram_tensor("bq2", [P, 1], F32, kind="ExternalInput")
    bk2_d = nc.dram_tensor("bk2", [P, 1], F32, kind="ExternalInput")
    bvg_d = nc.dram_tensor("bvg", [DIM, 1], F32, kind="ExternalInput")
    y_d = nc.dram_tensor("y", [DIM, HW], BF16, kind="ExternalOutput")

    with tile.TileContext(nc) as tc:
        with (
            tc.tile_pool(name="persist", bufs=1) as persist,
            tc.tile_pool(name="ptp", bufs=2) as ptp,
            tc.tile_pool(name="ystage", bufs=2) as ystage,
            tc.tile_pool(name="small", bufs=2) as small,
            tc.tile_pool(name="st_ps", bufs=2, space="PSUM") as st_ps,
            tc.tile_pool(name="out_ps", bufs=3, space="PSUM") as out_ps,
            tc.tile_pool(name="drb_ps", bufs=1, space="PSUM") as drb_ps,
        ):
            # ---- persistent tiles ----
            fo_f = persist.tile([P, 2, HW], FP8, tag="fof")
            sem_r = persist.tile([P, 2, HW], BF16, tag="semres")
            q2 = persist.tile([P, HW], BF16, tag="q2")
            k2 = persist.tile([P, HW], BF16, tag="k2")
            vt = persist.tile([P, PAIRS, 2, DIM], FP8, tag="vt")
            wq2s = persist.tile([P, 2, P], BF16, tag="wq2s")
            wk2s = persist.tile([P, 2, P], FP8, tag="wk2s")
            wvts = persist.tile([P, 2, DIM], FP8, tag="wvts")
            bq_s = persist.tile([P, 1], F32, tag="bq")
            bk_s = persist.tile([P, 1], F32, tag="bk")
            bvg_s = persist.tile([P, 2, 1], F32, tag="bvg")
            ones2 = persist.tile([P, 2, 16], FP8, tag="ones2")
            ones1 = persist.tile([1, P], BF16, tag="ones1")
            bm3 = persist.tile([P, 1], F32, tag="bm3")

            # ---- DMA issue: order matters per queue ----
            # scalar queue: tiny bias tensors only (the scalar dynamic queue
            # is slow for bulk; weights must ride the fast queues)
            nc.scalar.dma_start(bq_s[:], bq2_d[:])
            nc.scalar.dma_start(bk_s[:], bk2_d[:])
            nc.scalar.dma_start(bvg_s[:], bvg_d.ap().rearrange("(t p) o -> p t o", p=P))
            # sync/gpsimd queues carry weights then the bulk, interleaved by
            # t-half. Order: weights (gate every projection), foren block 0
            # (gates k-proj(0)), sem block 0 (gates q-proj(0)), rest of foren
            # (gates the later chunk-0 S pairs), rest of sem.
            nc.sync.dma_start(wk2s[:], wk2_d.ap().rearrange("(t p) o -> p t o", p=P))
            nc.gpsimd.dma_start(wq2s[:], wq2_d.ap().rearrange("(t p) o -> p t o", p=P))
            nc.gpsimd.dma_start(wvts[:], wvt_d.ap().rearrange("(t p) o -> p t o", p=P))
            # foren is fp8 (1MB total): front half on sync/gpsimd right after
            # the weights, back half on the scalar queue after the biases.
            nq = slice(2 * QP, 4 * QP)
            nc.scalar.dma_start(fo_f[:, 0, nq], foren_d[0:P, nq])
            nc.scalar.dma_start(fo_f[:, 1, nq], foren_d[P:2 * P, nq])
            nq = slice(0, 2 * QP)
            nc.sync.dma_start(fo_f[:, 0, nq], foren_d[0:P, nq])
            nc.gpsimd.dma_start(fo_f[:, 1, nq], foren_d[P:2 * P, nq])
            nc.sync.dma_start(sem_r[:, 0, 0:QP], sem_d[0:P, 0:QP])
            nc.gpsimd.dma_start(sem_r[:, 1, 0:QP], sem_d[P:2 * P, 0:QP])
            for pc_ in range(1, HW // QP):
                nq = slice(pc_ * QP, (pc_ + 1) * QP)
                nc.sync.dma_start(sem_r[:, 0, nq], sem_d[0:P, nq])
                nc.gpsimd.dma_start(sem_r[:, 1, nq], sem_d[P:2 * P, nq])

            nc.vector.memset(ones2[:], 1.0)
            nc.vector.memset(ones1[:], 1.0)
            nc.vector.memset(bm3[:], -3.0)

            pt = [ptp.tile([P, PAIRS, 2, NCH], FP8, tag="pt", name=f"pt{t}")
                  for t in range(2)]

            def s_pair(j, g, ns):
                """S^T matmul pair g of chunk j + exp eviction into pt."""
                st = st_ps.tile([P, 2, NCH], F32, tag="st", name="st")
                m0, m1 = 2 * g, 2 * g + 1
                nc.tensor.matmul(st[:, 0, :],
                                 k2[0:D4, m0 * P:(m0 + 1) * P], q2[0:D4, ns],
                                 start=True, stop=True, tile_position=(0, 0))
                nc.tensor.matmul(st[:, 1, :],
                                 k2[D4:P, m1 * P:(m1 + 1) * P], q2[D4:P, ns],
                                 start=True, stop=True, tile_position=(64, 0))
                if g in DVE_SET:
                    nc.vector.tensor_scalar(
                        pt[j % 2][:, g, :, :].bitcast(mybir.dt.uint8),
                        st[:], A8, B8, op0=ALU.mult, op1=ALU.add)
                else:
                    nc.scalar.activation(pt[j % 2][:, g, :, :], st[:],
                                         AF.Exp, bias=bm3[:], scale=0.125)

            # ---- phase 1: DMA-paced per-chunk pipeline ----
            for h in range(NCHUNKS):
                ns = slice(h * NCH, (h + 1) * NCH)
                # k-projection for chunk h: one fp8 DoubleRow matmul packs
                # both 128-channel halves (K=256); weights are host-scaled
                # by 16x to keep fp8 out of the subnormal range, undone by
                # the activation's free affine during the bias-add.
                pk = out_ps.tile([P, NCH], F32, tag="out", name="pk")
                nc.tensor.matmul(pk[:], wk2s[:], fo_f[:, :, ns],
                                 start=True, stop=True, perf_mode=DR)
                nc.scalar.activation(k2[:, ns], pk[:], AF.Identity,
                                     bias=bk_s[:], scale=1.0 / 16.0)
                if h == 0:
                    # q-projection chunk 0 (gates every chunk-0 S pair)
                    pq = out_ps.tile([P, NCH], F32, tag="out", name="pq")
                    for t in range(2):
                        nc.tensor.matmul(pq[:], wq2s[:, t, :],
                                         sem_r[:, t, 0:NCH],
                                         start=(t == 0), stop=(t == 1))
                    nc.vector.tensor_scalar_add(q2[:, 0:NCH], pq[:], bq_s[:])
                # v-projection: two m-tiles share one PSUM bank, one evac
                for mp in (4 * h, 4 * h + 2):
                    pv = out_ps.tile([P, NCH], F32, tag="out", name="pv")
                    for sl in range(2):
                        mi = mp + sl
                        c0 = h * NCH + (mi - 4 * h) * P
                        for t in range(2):
                            nc.tensor.matmul(pv[:, sl * DIM:(sl + 1) * DIM],
                                             fo_f[:, t, c0:c0 + P],
                                             wvts[:, t, :],
                                             start=(t == 0), stop=(t == 1))
                    if mp % 4 == 0:
                        nc.vector.tensor_scalar_mul(vt[:, mp // 2, :, :],
                                                    pv[:], 1.0 / 16.0)
                    else:
                        nc.scalar.mul(vt[:, mp // 2, :, :], pv[:], 1.0 / 16.0)
                # chunk-0 S pairs for the two m-tile pairs this chunk enables
                s_pair(0, 2 * h, slice(0, NCH))
                s_pair(0, 2 * h + 1, slice(0, NCH))
            for h in range(1, NCHUNKS):
                ns = slice(h * NCH, (h + 1) * NCH)
                pq = out_ps.tile([P, NCH], F32, tag="out", name="pq")
                for t in range(2):
                    nc.tensor.matmul(pq[:], wq2s[:, t, :], sem_r[:, t, ns],
                                     start=(t == 0), stop=(t == 1))
                nc.vector.tensor_scalar_add(q2[:, ns], pq[:], bq_s[:])

            # ---- phase 2: attention chunks ----
            # The PE instruction stream interleaves chunk j+1's S pairs with
            # chunk j's out-groups: each S matmul must wait for an st PSUM
            # bank (freed at exp cadence, ~1.1us), and with a strict-FIFO PE
            # queue an all-S-then-all-out order leaves the PE blocked at the
            # queue head for most of the exp span. Slotting one out-group
            # (3 ready-to-run DR matmuls, ~0.72us) between consecutive S
            # pairs keeps the PE busy while the st rotation catches up.
            # Per iteration j: (a) two plain DVE casts at the head of the
            # Vector queue evacuate chunk j-1's out accumulators to SBUF
            # (frees the PSUM buffers chunk j's first out-groups need within
            # ~1.4us of chunk start), (b) DVE computes chunk j-1's rinv +
            # normalize from SBUF, (c) the PE stream interleaves chunk j's
            # out-groups with chunk j+1's S pairs (each S matmul waits on an
            # st PSUM bank freed at exp cadence; slotting one ready-to-run
            # out-group between S pairs keeps the strict-FIFO PE queue busy).
            order = ([g for g in range(PAIRS) if g not in DVE_SET]
                     + list(DVE_SET))
            prev = None  # (out0, out1, den, ns) of chunk j-1

            def finish_chunk(prev):
                out0p, out1p, drb, nsp = prev
                out0s = ystage.tile([P, NCH], BF16, tag="o0s", name="out0s")
                out1s = ystage.tile([P, NCH], BF16, tag="o1s", name="out1s")
                nc.vector.tensor_copy(out0s[:], out0p[:])
                nc.vector.tensor_copy(out1s[:], out1p[:])
                rrf = small.tile([1, NCH], F32, tag="rrf", name="rrf")
                nc.vector.reciprocal_approx_fast(rrf[:], drb[0:1, :])
                rr = small.tile([1, NCH], BF16, tag="rr", name="rr")
                nc.vector.tensor_copy(rr[:], rrf[:])
                # rinv broadcast reuses the den bank: the rank-1 matmul's
                # write of the full [P, NCH] region overlaps den's [0:1, :]
                # slice, so Tile orders it after the reciprocal's read, and
                # orders the next chunk's den accumulation after the rb cast.
                nc.tensor.matmul(drb[:], ones1[:], rr[:], start=True, stop=True)
                rb = small.tile([P, NCH], BF16, tag="rbs", name="rb")
                nc.vector.tensor_copy(rb[:], drb[:])
                yst = ystage.tile([P, 2, NCH], BF16, tag="yst", name="yst")
                for ct, outp in ((0, out0s), (1, out1s)):
                    nc.vector.tensor_tensor(yst[:, ct, :], outp[:], rb[:],
                                            op=mybir.AluOpType.mult)
                    nc.vector.scalar_tensor_tensor(yst[:, ct, :], yst[:, ct, :],
                                                   bvg_s[:, ct, :],
                                                   sem_r[:, ct, nsp],
                                                   op0=mybir.AluOpType.add,
                                                   op1=mybir.AluOpType.add)
                    nc.sync.dma_start(y_d[ct * P:(ct + 1) * P, nsp],
                                      yst[:, ct, :])

            for j in range(NCHUNKS):
                ns = slice(j * NCH, (j + 1) * NCH)
                ns1 = slice((j + 1) * NCH, (j + 2) * NCH)
                out0 = out_ps.tile([P, NCH], F32, tag="out", name="out0")
                out1 = out_ps.tile([P, NCH], F32, tag="out", name="out1")
                drb = drb_ps.tile([P, NCH], F32, tag="drb", name="drb")

                def out_group(idx):
                    g = order[idx]
                    pslice = pt[j % 2][:, g, :, :]
                    first, last = idx == 0, idx == PAIRS - 1
                    nc.tensor.matmul(out0[:], vt[:, g, :, 0:P], pslice,
                                     start=first, stop=last, perf_mode=DR)
                    nc.tensor.matmul(out1[:], vt[:, g, :, P:DIM], pslice,
                                     start=first, stop=last, perf_mode=DR)
                    nc.tensor.matmul(drb[0:1, :], ones2[:, :, 0:1], pslice,
                                     start=first, stop=last, perf_mode=DR)

                # S pairs clustered in twos: each out<->S transition on the
                # PE costs ~100ns (the row-packed S pair blocks the next
                # LDWEIGHTS pull-ahead), so fewer, larger clusters beat a
                # strict 1:1 interleave.
                HEAD = 2
                if j + 1 < NCHUNKS:
                    for g in range(HEAD):
                        s_pair(j + 1, g, ns1)
                        if g == 1 and prev is not None:
                            finish_chunk(prev)
                    for g in range(HEAD, PAIRS, 2):
                        out_group(g - HEAD)
                        out_group(g - HEAD + 1)
                        s_pair(j + 1, g, ns1)
                        s_pair(j + 1, g + 1, ns1)
                    for idx in range(PAIRS - HEAD, PAIRS):
                        out_group(idx)
                else:
                    if prev is not None:
                        finish_chunk(prev)
                    for idx in range(PAIRS):
                        out_group(idx)
                prev = (out0, out1, drb, ns)
            finish_chunk(prev)

    nc.compile()
    return nc


def _get_program():
    if "nc" not in _CACHE:
        _CACHE["nc"] = _build_program()
    return _CACHE["nc"]


def kernel(sem, foren, Wq, bq, Wk, bk, Wv, bv, gamma):
    BF = ml_dtypes.bfloat16
    sem = np.asarray(sem, dtype=np.float32)
    foren = np.asarray(foren, dtype=np.float32)
    F8 = ml_dtypes.float8_e4m3
    wqt = np.asarray(Wq, np.float32).T          # [256, 64]
    wkt = np.asarray(Wk, np.float32).T
    g = float(np.asarray(gamma, np.float32).reshape(()))
    # k/v weights ride in fp8, pre-scaled by 16 so their ~N(0, 1/256)
    # entries land in fp8e4m3's normal range; the device undoes the 16x
    # during PSUM evacuation.
    wvtg = np.ascontiguousarray(16.0 * g * np.asarray(Wv, np.float32).T).astype(F8)
    wq2 = np.ascontiguousarray(np.concatenate([wqt, wqt], axis=1)).astype(BF)
    wk2 = np.ascontiguousarray(16.0 * np.concatenate([wkt, wkt], axis=1)).astype(F8)
    bqv = np.asarray(bq, np.float32).reshape(D4, 1)
    bkv = np.asarray(bk, np.float32).reshape(D4, 1)
    bq2 = np.ascontiguousarray(np.tile(bqv, (2, 1)))
    bk2 = np.ascontiguousarray(np.tile(bkv, (2, 1)))
    bvg = np.ascontiguousarray(g * np.asarray(bv, np.float32).reshape(DIM, 1))

    B = sem.shape[0]
    assert B == N_CORES, f"expected batch {N_CORES}, got {B}"

    in_maps = []
    for i in range(N_CORES):
        in_maps.append({
            "sem_b": np.ascontiguousarray(sem[i].reshape(DIM, HW)).astype(BF),
            "foren_b": np.ascontiguousarray(foren[i].reshape(DIM, HW)).astype(F8),
            "wq2": wq2, "wk2": wk2, "wvt": wvtg,
            "bq2": bq2, "bk2": bk2, "bvg": bvg,
        })

    nc = _get_program()
    res = run_bass_kernel_spmd(nc, in_maps, list(range(N_CORES)), trace=TRACE)
    if TRACE:
        _CACHE["last_exec_time_ns"] = res.exec_time_ns
        _CACHE["last_results"] = res

    H = int(np.sqrt(HW))
    out = np.stack([np.asarray(res.results[i]["y"]).astype(np.float32)
                    .reshape(DIM, H, H) for i in range(N_CORES)])
    return out


# revision 34
# speedup vs baseline: 1.0320x; 1.0124x over previous
"""Cross-attention kernel for Trainium2, data-parallel over batch across 8 NeuronCores.

Reference computation (per batch element b):
    q = Wq @ sem_b + bq   [64, 4096]   (1x1 conv == per-pixel linear)
    k = Wk @ foren_b + bk [64, 4096]
    v = Wv @ foren_b + bv [256, 4096]
    S = (q^T k) / 8                [4096 (n), 4096 (m)]
    P = softmax_m(S)
    out = v @ P^T                  [256, 4096]
    y = sem_b + gamma * out

Kernel structure (bf16 I/O, DMA-paced phase 1, interleaved phase 2):
  - Host casts sem/foren/weights to bf16 (halves input DMA to ~4.2MB) and
    folds gamma into Wv^T and bv. Output y is written bf16 and upcast on host.
  - q/k weights are column-duplicated on the host ([W^T|W^T] -> M=128
    stationary) so projections avoid the M=64 half-rate path AND land q/k
    duplicated in both partition halves, as the 2-row-packed S matmul needs.
  - Phase 1 is a per-chunk pipeline paced by DMA: as foren chunk h lands,
    k-proj(h), v-proj(h) and the two chunk-0 S^T pairs (2h, 2h+1) run
    immediately (q2 chunk 0 is projected first from an early sem block), so
    the exp engines start ~16us into the kernel instead of after all loads.
    PSUM evacuations are split across ScalarE (k2 bias-add via
    Identity-activation, half the vt copies) and VectorE (q2 bias-add, rest).
  - S^T tiles ([m-tile, n-chunk], m on partitions) via bf16 k2/q2: pairs of
    K=64 matmuls packed in the two PE row halves (tile_position) run
    concurrently (~one N=512 stream per pair).
  - exp: Exp(0.125*st - 3) -> fp8e4m3 pt tiles laid out [k, pair, slot, n].
    The -3 shift cancels in softmax (denominator uses the same shift).
    10 pairs/chunk on ScalarE (activation); 6 on VectorE via a single
    tensor_scalar producing fp8e4 BIT PATTERNS directly (Schraudolph in the
    3-bit-mantissa domain, ~7% worst case -- same order as exp->fp8 RNE).
  - out^[c, n] = sum_m vt[m, c] * P~[m, n] via fp8 DoubleRow matmuls:
    stationary vt [Ki=128, 2, 128] (two m-tiles packed per weight load),
    moving pt [Ki=128, 2, 512] -> 2x bf16 rate (216ns/matmul measured).
    A third "ones" group accumulates the softmax denominator den[1, n] into
    partition 0 of a bank shared with the rinv broadcast (Tile's slice
    tracking orders the overlap), freeing a PSUM bank for a 3rd out buffer.
  - Phase 2 PE stream interleaves chunk j+1's S pairs (gated by the 2-buffer
    st rotation at exp cadence) with chunk j's always-ready out-groups in a
    2:2 pattern, so the strict-FIFO PE queue never idles at a blocked S
    matmul. DVE_SET = even pairs keeps each st buffer's exp chain on one
    engine. Chunk j-1's out accumulators are evacuated to SBUF by two DVE
    casts at the head of the chunk's Vector queue; normalize + residual run
    from SBUF (y = (sem + gamma*bv) + gamma*out*rinv). No PE transposes.
"""

import os
import sys

for _p in ("/opt/trn_rl_repo",):
    if _p not in sys.path and os.path.isdir(_p):
        sys.path.append(_p)

import numpy as np
import ml_dtypes

import concourse.bass as bass
import concourse.tile as tile
from concourse import bacc, mybir
from concourse.bass_utils import run_bass_kernel_spmd

N_CORES = 8
DIM = 256
D4 = 64
HW = 4096
P = 128
NCH = 512             # columns per chunk
NCHUNKS = HW // NCH   # 8
MT = HW // P          # 32 m-tiles
PAIRS = MT // 2       # 16 DoubleRow pairs
QP = 1024             # DMA block width (2KB bf16 lines)

F32 = mybir.dt.float32
BF16 = mybir.dt.bfloat16
FP8 = mybir.dt.float8e4

TRACE = False
_CACHE = {}

# fp8e4m3-bits Schraudolph exp for the DVE offload pairs (single op):
# u8 = round(A8*st + B8) are the bit patterns of fp8e4 exp(0.125*st - 3)
# to within ~7% -- same worst-case as exact-exp-then-fp8-RNE would give
# after the Schraudolph fp16 detour, but one DVE instruction instead of
# two. (3-bit mantissa -> exponent multiplier 8; bias 7 and the -3 shift
# land all values in the normal range, bits 12..30.)
A8 = 8 * 0.125 / 0.6931471805599453   # 8*log2(e)*0.125
B8 = 21.005  # 8*(7 - 3*log2(e)) - 0.367, fitted vs fp8 decode (RNE convert)
# pairs per chunk evaluated on VectorE. With 2 st PSUM buffers, S pair g
# waits on exp(g-2): buffer A serves even pairs, buffer B odd pairs. An
# all-even DVE set keeps each buffer's exp chain on (mostly) one engine,
# so the two chains drain independently instead of cross-stalling.
DVE_SET = (2, 4, 6, 8, 10, 12)


def _build_program():
    AF = mybir.ActivationFunctionType
    ALU = mybir.AluOpType
    DR = mybir.MatmulPerfMode.DoubleRow

    nc = bacc.Bacc("TRN2", target_bir_lowering=False, debug=False,
                   num_devices=N_CORES)

    sem_d = nc.dram_tensor("sem_b", [DIM, HW], BF16, kind="ExternalInput")
    foren_d = nc.dram_tensor("foren_b", [DIM, HW], FP8, kind="ExternalInput")
    wq2_d = nc.dram_tensor("wq2", [DIM, P], BF16, kind="ExternalInput")
    wk2_d = nc.dram_tensor("wk2", [DIM, P], FP8, kind="ExternalInput")
    wvt_d = nc.dram_tensor("wvt", [DIM, DIM], FP8, kind="ExternalInput")
    bq2_d = nc.dram_tensor("bq2", [P, 1], F32, kind="ExternalInput")
    bk2_d = nc.dram_tensor("bk2", [P, 1], F32, kind="ExternalInput")
    bvg_d = nc.dram_tensor("bvg", [DIM, 1], F32, kind="ExternalInput")
    y_d = nc.dram_tensor("y", [DIM, HW], BF16, kind="ExternalOutput")

    with tile.TileContext(nc) as tc:
        with (
            tc.tile_pool(name="persist", bufs=1) as persist,
            tc.tile_pool(name="ptp", bufs=2) as ptp,
            tc.tile_pool(name="ystage", bufs=2) as ystage,
            tc.tile_pool(name="small", bufs=2) as small,
            tc.tile_pool(name="st_ps", bufs=2, space="PSUM") as st_ps,
            tc.tile_pool(name="out_ps", bufs=3, space="PSUM") as out_ps,
            tc.tile_pool(name="drb_ps", bufs=1, space="PSUM") as drb_ps,
        ):
            # ---- persistent tiles ----
            fo_f = persist.tile([P, 2, HW], FP8, tag="fof")
            sem_r = persist.tile([P, 2, HW], BF16, tag="semres")
            q2 = persist.tile([P, HW], BF16, tag="q2")
            k2 = persist.tile([P, HW], BF16, tag="k2")
            vt = persist.tile([P, PAIRS, 2, DIM], FP8, tag="vt")
            wq2s = persist.tile([P, 2, P], BF16, tag="wq2s")
            wk2s = persist.tile([P, 2, P], FP8, tag="wk2s")
            wvts = persist.tile([P, 2, DIM], FP8, tag="wvts")
            bq_s = persist.tile([P, 1], F32, tag="bq")
            bk_s = persist.tile([P, 1], F32, tag="bk")
            bvg_s = persist.tile([P, 2, 1], F32, tag="bvg")
            ones2 = persist.tile([P, 2, 16], FP8, tag="ones2")
            ones1 = persist.tile([1, P], BF16, tag="ones1")
            bm3 = persist.tile([P, 1], F32, tag="bm3")

            # ---- DMA issue: order matters per queue ----
            # scalar queue: tiny bias tensors only (the scalar dynamic queue
            # is slow for bulk; weights must ride the fast queues)
            nc.scalar.dma_start(bq_s[:], bq2_d[:])
            nc.scalar.dma_start(bk_s[:], bk2_d[:])
            nc.scalar.dma_start(bvg_s[:], bvg_d.ap().rearrange("(t p) o -> p t o", p=P))
            # sync/gpsimd queues carry weights then the bulk, interleaved by
            # t-half. Order: weights (gate every projection), foren block 0
            # (gates k-proj(0)), sem block 0 (gates q-proj(0)), rest of foren
            # (gates the later chunk-0 S pairs), rest of sem.
            nc.sync.dma_start(wk2s[:], wk2_d.ap().rearrange("(t p) o -> p t o", p=P))
            nc.gpsimd.dma_start(wq2s[:], wq2_d.ap().rearrange("(t p) o -> p t o", p=P))
            nc.gpsimd.dma_start(wvts[:], wvt_d.ap().rearrange("(t p) o -> p t o", p=P))
            # foren is fp8 (1MB total): front half on sync/gpsimd right after
            # the weights, back half on the scalar queue after the biases.
            nq = slice(2 * QP, 4 * QP)
            nc.scalar.dma_start(fo_f[:, 0, nq], foren_d[0:P, nq])
            nc.scalar.dma_start(fo_f[:, 1, nq], foren_d[P:2 * P, nq])
            for pc_ in (0, 1):
                nq = slice(pc_ * QP, (pc_ + 1) * QP)
                nc.sync.dma_start(fo_f[:, 0, nq], foren_d[0:P, nq])
                nc.gpsimd.dma_start(fo_f[:, 1, nq], foren_d[P:2 * P, nq])
            nc.sync.dma_start(sem_r[:, 0, 0:QP], sem_d[0:P, 0:QP])
            nc.gpsimd.dma_start(sem_r[:, 1, 0:QP], sem_d[P:2 * P, 0:QP])
            for pc_ in range(1, HW // QP):
                nq = slice(pc_ * QP, (pc_ + 1) * QP)
                nc.sync.dma_start(sem_r[:, 0, nq], sem_d[0:P, nq])
                nc.gpsimd.dma_start(sem_r[:, 1, nq], sem_d[P:2 * P, nq])

            nc.vector.memset(ones2[:], 1.0)
            nc.vector.memset(ones1[:], 1.0)
            nc.vector.memset(bm3[:], -3.0)

            pt = [ptp.tile([P, PAIRS, 2, NCH], FP8, tag="pt", name=f"pt{t}")
                  for t in range(2)]

            def s_pair(j, g, ns):
                """S^T matmul pair g of chunk j + exp eviction into pt."""
                st = st_ps.tile([P, 2, NCH], F32, tag="st", name="st")
                m0, m1 = 2 * g, 2 * g + 1
                nc.tensor.matmul(st[:, 0, :],
                                 k2[0:D4, m0 * P:(m0 + 1) * P], q2[0:D4, ns],
                                 start=True, stop=True, tile_position=(0, 0))
                nc.tensor.matmul(st[:, 1, :],
                                 k2[D4:P, m1 * P:(m1 + 1) * P], q2[D4:P, ns],
                                 start=True, stop=True, tile_position=(64, 0))
                if g in DVE_SET:
                    nc.vector.tensor_scalar(
                        pt[j % 2][:, g, :, :].bitcast(mybir.dt.uint8),
                        st[:], A8, B8, op0=ALU.mult, op1=ALU.add)
                else:
                    nc.scalar.activation(pt[j % 2][:, g, :, :], st[:],
                                         AF.Exp, bias=bm3[:], scale=0.125)

            # ---- phase 1: DMA-paced per-chunk pipeline ----
            for h in range(NCHUNKS):
                ns = slice(h * NCH, (h + 1) * NCH)
                # k-projection for chunk h: one fp8 DoubleRow matmul packs
                # both 128-channel halves (K=256); weights are host-scaled
                # by 16x to keep fp8 out of the subnormal range, undone by
                # the activation's free affine during the bias-add.
                pk = out_ps.tile([P, NCH], F32, tag="out", name="pk")
                nc.tensor.matmul(pk[:], wk2s[:], fo_f[:, :, ns],
                                 start=True, stop=True, perf_mode=DR)
                nc.scalar.activation(k2[:, ns], pk[:], AF.Identity,
                                     bias=bk_s[:], scale=1.0 / 16.0)
                if h == 0:
                    # q-projection chunk 0 (gates every chunk-0 S pair)
                    pq = out_ps.tile([P, NCH], F32, tag="out", name="pq")
                    for t in range(2):
                        nc.tensor.matmul(pq[:], wq2s[:, t, :],
                                         sem_r[:, t, 0:NCH],
                                         start=(t == 0), stop=(t == 1))
                    nc.vector.tensor_scalar_add(q2[:, 0:NCH], pq[:], bq_s[:])
                # v-projection: two m-tiles share one PSUM bank, one evac
                for mp in (4 * h, 4 * h + 2):
                    pv = out_ps.tile([P, NCH], F32, tag="out", name="pv")
                    for sl in range(2):
                        mi = mp + sl
                        c0 = h * NCH + (mi - 4 * h) * P
                        for t in range(2):
                            nc.tensor.matmul(pv[:, sl * DIM:(sl + 1) * DIM],
                                             fo_f[:, t, c0:c0 + P],
                                             wvts[:, t, :],
                                             start=(t == 0), stop=(t == 1))
                    if mp % 4 == 0:
                        nc.vector.tensor_scalar_mul(vt[:, mp // 2, :, :],
                                                    pv[:], 1.0 / 16.0)
                    else:
                        nc.scalar.mul(vt[:, mp // 2, :, :], pv[:], 1.0 / 16.0)
                # chunk-0 S pairs for the two m-tile pairs this chunk enables
                s_pair(0, 2 * h, slice(0, NCH))
                s_pair(0, 2 * h + 1, slice(0, NCH))
            for h in range(1, NCHUNKS):
                ns = slice(h * NCH, (h + 1) * NCH)
                pq = out_ps.tile([P, NCH], F32, tag="out", name="pq")
                for t in range(2):
                    nc.tensor.matmul(pq[:], wq2s[:, t, :], sem_r[:, t, ns],
                                     start=(t == 0), stop=(t == 1))
                nc.vector.tensor_scalar_add(q2[:, ns], pq[:], bq_s[:])

            # ---- phase 2: attention chunks ----
            # The PE instruction stream interleaves chunk j+1's S pairs with
            # chunk j's out-groups: each S matmul must wait for an st PSUM
            # bank (freed at exp cadence, ~1.1us), and with a strict-FIFO PE
            # queue an all-S-then-all-out order leaves the PE blocked at the
            # queue head for most of the exp span. Slotting one out-group
            # (3 ready-to-run DR matmuls, ~0.72us) between consecutive S
            # pairs keeps the PE busy while the st rotation catches up.
            # Per iteration j: (a) two plain DVE casts at the head of the
            # Vector queue evacuate chunk j-1's out accumulators to SBUF
            # (frees the PSUM buffers chunk j's first out-groups need within
            # ~1.4us of chunk start), (b) DVE computes chunk j-1's rinv +
            # normalize from SBUF, (c) the PE stream interleaves chunk j's
            # out-groups with chunk j+1's S pairs (each S matmul waits on an
            # st PSUM bank freed at exp cadence; slotting one ready-to-run
            # out-group between S pairs keeps the strict-FIFO PE queue busy).
            order = ([g for g in range(PAIRS) if g not in DVE_SET]
                     + list(DVE_SET))
            prev = None  # (out0, out1, den, ns) of chunk j-1

            def finish_chunk(prev):
                out0p, out1p, drb, nsp = prev
                out0s = ystage.tile([P, NCH], BF16, tag="o0s", name="out0s")
                out1s = ystage.tile([P, NCH], BF16, tag="o1s", name="out1s")
                nc.vector.tensor_copy(out0s[:], out0p[:])
                nc.vector.tensor_copy(out1s[:], out1p[:])
                rrf = small.tile([1, NCH], F32, tag="rrf", name="rrf")
                nc.vector.reciprocal_approx_fast(rrf[:], drb[0:1, :])
                rr = small.tile([1, NCH], BF16, tag="rr", name="rr")
                nc.vector.tensor_copy(rr[:], rrf[:])
                # rinv broadcast reuses the den bank: the rank-1 matmul's
                # write of the full [P, NCH] region overlaps den's [0:1, :]
                # slice, so Tile orders it after the reciprocal's read, and
                # orders the next chunk's den accumulation after the rb cast.
                nc.tensor.matmul(drb[:], ones1[:], rr[:], start=True, stop=True)
                rb = small.tile([P, NCH], BF16, tag="rbs", name="rb")
                nc.vector.tensor_copy(rb[:], drb[:])
                yst = ystage.tile([P, 2, NCH], BF16, tag="yst", name="yst")
                for ct, outp in ((0, out0s), (1, out1s)):
                    nc.vector.tensor_tensor(yst[:, ct, :], outp[:], rb[:],
                                            op=mybir.AluOpType.mult)
                    nc.vector.scalar_tensor_tensor(yst[:, ct, :], yst[:, ct, :],
                                                   bvg_s[:, ct, :],
                                                   sem_r[:, ct, nsp],
                                                   op0=mybir.AluOpType.add,
                                                   op1=mybir.AluOpType.add)
                    nc.sync.dma_start(y_d[ct * P:(ct + 1) * P, nsp],
                                      yst[:, ct, :])

            for j in range(NCHUNKS):
                ns = slice(j * NCH, (j + 1) * NCH)
                ns1 = slice((j + 1) * NCH, (j + 2) * NCH)
                out0 = out_ps.tile([P, NCH], F32, tag="out", name="out0")
                out1 = out_ps.tile([P, NCH], F32, tag="out", name="out1")
                drb = drb_ps.tile([P, NCH], F32, tag="drb", name="drb")

                def out_group(idx):
                    g = order[idx]
                    pslice = pt[j % 2][:, g, :, :]
                    first, last = idx == 0, idx == PAIRS - 1
                    nc.tensor.matmul(out0[:], vt[:, g, :, 0:P], pslice,
                                     start=first, stop=last, perf_mode=DR)
                    nc.tensor.matmul(out1[:], vt[:, g, :, P:DIM], pslice,
                                     start=first, stop=last, perf_mode=DR)
                    nc.tensor.matmul(drb[0:1, :], ones2[:, :, 0:1], pslice,
                                     start=first, stop=last, perf_mode=DR)

                # S pairs clustered in twos: each out<->S transition on the
                # PE costs ~100ns (the row-packed S pair blocks the next
                # LDWEIGHTS pull-ahead), so fewer, larger clusters beat a
                # strict 1:1 interleave.
                HEAD = 2
                if j + 1 < NCHUNKS:
                    for g in range(HEAD):
                        s_pair(j + 1, g, ns1)
                        if g == 1 and prev is not None:
                            finish_chunk(prev)
                    for g in range(HEAD, PAIRS, 2):
                        out_group(g - HEAD)
                        out_group(g - HEAD + 1)
                        s_pair(j + 1, g, ns1)
                        s_pair(j + 1, g + 1, ns1)
                    for idx in range(PAIRS - HEAD, PAIRS):
                        out_group(idx)
                else:
                    # Last chunk: after two out-pairs (so the PE isn't blocked
                    # at the FIFO head while the previous chunk's rb-cast
                    # releases the shared den/rb bank), run ALL den matmuls in
                    # one pass (single ones-weights load). The rinv chain then
                    # overlaps the remaining out0/out1 accumulation instead of
                    # trailing it, shortening the kernel tail by ~3us.
                    if prev is not None:
                        finish_chunk(prev)

                    def out_pair(idx, first, last):
                        g = order[idx]
                        pslice = pt[j % 2][:, g, :, :]
                        nc.tensor.matmul(out0[:], vt[:, g, :, 0:P], pslice,
                                         start=first, stop=last, perf_mode=DR)
                        nc.tensor.matmul(out1[:], vt[:, g, :, P:DIM], pslice,
                                         start=first, stop=last, perf_mode=DR)

                    out_pair(0, True, False)
                    out_pair(1, False, False)
                    for idx in range(PAIRS):
                        nc.tensor.matmul(drb[0:1, :], ones2[:, :, 0:1],
                                         pt[j % 2][:, order[idx], :, :],
                                         start=idx == 0, stop=idx == PAIRS - 1,
                                         perf_mode=DR)
                    rrf7 = small.tile([1, NCH], F32, tag="rrf", name="rrf7")
                    nc.vector.reciprocal_approx_fast(rrf7[:], drb[0:1, :])
                    rr7 = small.tile([1, NCH], BF16, tag="rr", name="rr7")
                    nc.vector.tensor_copy(rr7[:], rrf7[:])
                    for idx in range(2, PAIRS):
                        out_pair(idx, False, idx == PAIRS - 1)
                        if idx == 4:
                            nc.tensor.matmul(drb[:], ones1[:], rr7[:],
                                             start=True, stop=True)
                    rb7 = small.tile([P, NCH], BF16, tag="rbs", name="rb7")
                    nc.vector.tensor_copy(rb7[:], drb[:])
                    yst7 = ystage.tile([P, 2, NCH], BF16, tag="yst", name="yst7")
                    for ct, outp in ((0, out0), (1, out1)):
                        nc.vector.tensor_tensor(yst7[:, ct, :], outp[:],
                                                rb7[:], op=mybir.AluOpType.mult)
                        nc.vector.scalar_tensor_tensor(
                            yst7[:, ct, :], yst7[:, ct, :], bvg_s[:, ct, :],
                            sem_r[:, ct, ns],
                            op0=mybir.AluOpType.add, op1=mybir.AluOpType.add)
                        nc.sync.dma_start(y_d[ct * P:(ct + 1) * P, ns],
                                          yst7[:, ct, :])
                prev = (out0, out1, drb, ns)

    nc.compile()
    return nc


def _get_program():
    if "nc" not in _CACHE:
        _CACHE["nc"] = _build_program()
    return _CACHE["nc"]


def kernel(sem, foren, Wq, bq, Wk, bk, Wv, bv, gamma):
    BF = ml_dtypes.bfloat16
    sem = np.asarray(sem, dtype=np.float32)
    foren = np.asarray(foren, dtype=np.float32)
    F8 = ml_dtypes.float8_e4m3
    wqt = np.asarray(Wq, np.float32).T          # [256, 64]
    wkt = np.asarray(Wk, np.float32).T
    g = float(np.asarray(gamma, np.float32).reshape(()))
    # k/v weights ride in fp8, pre-scaled by 16 so their ~N(0, 1/256)
    # entries land in fp8e4m3's normal range; the device undoes the 16x
    # during PSUM evacuation.
    wvtg = np.ascontiguousarray(16.0 * g * np.asarray(Wv, np.float32).T).astype(F8)
    wq2 = np.ascontiguousarray(np.concatenate([wqt, wqt], axis=1)).astype(BF)
    wk2 = np.ascontiguousarray(16.0 * np.concatenate([wkt, wkt], axis=1)).astype(F8)
    bqv = np.asarray(bq, np.float32).reshape(D4, 1)
    bkv = np.asarray(bk, np.float32).reshape(D4, 1)
    bq2 = np.ascontiguousarray(np.tile(bqv, (2, 1)))
    bk2 = np.ascontiguousarray(np.tile(bkv, (2, 1)))
    bvg = np.ascontiguousarray(g * np.asarray(bv, np.float32).reshape(DIM, 1))

    B = sem.shape[0]
    assert B == N_CORES, f"expected batch {N_CORES}, got {B}"

    in_maps = []
    for i in range(N_CORES):
        in_maps.append({
            "sem_b": np.ascontiguousarray(sem[i].reshape(DIM, HW)).astype(BF),
            "foren_b": np.ascontiguousarray(foren[i].reshape(DIM, HW)).astype(F8),
            "wq2": wq2, "wk2": wk2, "wvt": wvtg,
            "bq2": bq2, "bk2": bk2, "bvg": bvg,
        })

    nc = _get_program()
    res = run_bass_kernel_spmd(nc, in_maps, list(range(N_CORES)), trace=TRACE)
    if TRACE:
        _CACHE["last_exec_time_ns"] = res.exec_time_ns
        _CACHE["last_results"] = res

    H = int(np.sqrt(HW))
    out = np.stack([np.asarray(res.results[i]["y"]).astype(np.float32)
                    .reshape(DIM, H, H) for i in range(N_CORES)])
    return out
